# revision 1
# baseline (speedup 1.0000x reference)
"""AttentionWide (t=2048, e=512, h=8) on 8 TRN2 NeuronCores.

Tensor-parallel over heads: core i owns head i (columns i*512:(i+1)*512 of
Wk/Wq/Wv, rows i*512:(i+1)*512 of Wu).  Each core computes its head's
attention and the partial unifyheads product; chunked ReduceScatters sum the
partials across cores, each core returning row-shards of the final output.

Weight folding (host-side, exact algebra — the head dim equals emb here so
no information is lost):
    scores = q k^T = (y Wq)(x Wk)^T = y (Wq Wk^T) x^T
      ->  G  = Wk Wq^T   [e, e]   (host)
          g  = x G                 (device)
          scoresT[tk, tq] = g y^T  (device)
    out = attn @ v @ Wu = attn @ (x Wv Wu)
      ->  W2 = Wv Wu     [e, e]   (host)
          vW = x W2                (device)
This removes two of the four projection matmul groups.

Device compute in bf16 with fp32 PSUM accumulation, in "transposed"
layouts so no on-device transposes are needed (x/y transposed on host):
    gT = G^T @ xT            lhsT=G (natural),  rhs=xT       [e, t]
    vW = xT^T @ W2           lhsT=xT slices,    rhs=W2       [t, e]
    scoresT[tk,tq] = gT^T yT (fp8e4 DoubleRow: 2x PE throughput; gT and
                              yT quantized to e4m3, ~1.2% final rel err)
    expT = exp(scoresT * e^-0.5)  (softmax w/o max-subtraction; |scores|<~2)
    out[tq,:] = (expT^T @ vW) / denom[tq]   lhsT=expT slices, 512-wide rhs
    denom via DVE running sum S of the 16 exp tiles + PE transpose of
    S 128-col slices (identity matmul) + DVE free-dim reduce.
"""

import os
import numpy as np
import ml_dtypes

T, E, H = 2048, 512, 8
NCORES = 8
TB = 512          # matmul moving-operand block (free dim; one fp32 PSUM bank)
NE = E // 128     # 4  partition tiles of the emb dim
NT = T // 128     # 16 partition tiles of the seq dim
NB = T // TB      # 4  seq blocks
# ReduceScatter chunks (rows per chunk, sum = T).  One chunk per seq block:
# the RS chain is near-saturated at the tail (~6.5us fixed + ~13us/MB per
# op), and this split measured best (177us median, tight spread) vs
# [768,768,512] (179.5us, wide spread) and [1024,512,384,128] (190us).
CHUNKS = [512, 512, 512, 512]
NCH = len(CHUNKS)
M0 = 1.02         # mean shift for the fp8 attn@v matmul (exp values ~N(1.02,0.21))

_cache = {}
last_result = None


def _build_nc():
    from concourse import bacc, tile
    from concourse.bass import mybir

    bf16 = mybir.dt.bfloat16
    f16 = mybir.dt.float16
    f32 = mybir.dt.float32
    f8 = mybir.dt.float8e4

    nc = bacc.Bacc(
        "TRN2", target_bir_lowering=False, debug=False, num_devices=NCORES
    )

    xT = nc.dram_tensor("xT", [E, T], bf16, kind="ExternalInput")
    yT = nc.dram_tensor("yT", [E, T], f8, kind="ExternalInput")
    gw = nc.dram_tensor("gw", [E, E], bf16, kind="ExternalInput")   # Wk Wq^T
    w2 = nc.dram_tensor("w2", [E, E], bf16, kind="ExternalInput")   # Wv Wu
    ident = nc.dram_tensor("ident", [128, 128], f32, kind="ExternalInput")
    # csb[p, e] = M0 * colsum(vW)[e] = M0 * (sum_t x[t,:]) @ W2, replicated
    # across partitions on the host: the mean-shifted attn@v needs
    # out[tq,e] = (delta @ vW)[tq,e] + M0*colsum(vW)[e], then * 1/denom.
    csb = nc.dram_tensor("csb", [128, E], f32, kind="ExternalInput")
    # fp16 reduction payload: the partials are ~N(0, 0.1^2) so fp16 keeps
    # ~3 more mantissa bits than bf16 and halves the collective bytes.
    out_exts = [
        nc.dram_tensor(f"out{c}", [CHUNKS[c] // NCORES, E], f16, kind="ExternalOutput")
        for c in range(NCH)
    ]

    with tile.TileContext(nc) as tc:
        with (
            tc.tile_pool(name="persist", bufs=1) as persist,
            tc.tile_pool(name="work", bufs=4) as work,
            tc.tile_pool(name="expp", bufs=32) as expp,
            tc.tile_pool(name="psum", bufs=2, space="PSUM") as psum_pool,
            tc.tile_pool(name="dram", bufs=1, space="DRAM") as dram,
        ):
            def alloc_rows(prefix, n):
                return [
                    persist.tile(
                        [128, n], bf16, tag=f"{prefix}{j}", name=f"{prefix}{j}"
                    )
                    for j in range(NE)
                ]

            xT_sb = alloc_rows("xTs", T)
            gw_sb = alloc_rows("gws", E)
            w2_sb = alloc_rows("w2s", E)
            # fp8 wide tiles: free dims = (e-slice, t) so DoubleRow matmuls
            # can take [128, 2, n] k-pair slices.
            yT_sb = persist.tile([128, NE, T], f8, tag="yTs", name="yTs")
            gT_sb = persist.tile([128, NE, T], f8, tag="gTs", name="gTs")
            ident_sb = persist.tile([128, 128], f32, tag="ident", name="ident")
            csb_sb = persist.tile([128, E], f32, tag="csb", name="csb")

            # DMA order = need order: gw, xT col-chunk 0, w2, rest of xT,
            # then yT.  Column-chunked so the first projection matmuls can
            # start after ~1MB has landed.
            # The first 8 dma_starts land on the first 8 hardware queues,
            # which boot ~3-6us into the NEFF (queues 8-15 only by ~9us) —
            # so issue the critical gw + xT block-0 loads unsplit, first.
            for j in range(NE):
                nc.sync.dma_start(gw_sb[j][:], gw[j * 128 : (j + 1) * 128, :])
                nc.sync.dma_start(
                    xT_sb[j][:, 0:TB], xT[j * 128 : (j + 1) * 128, 0:TB]
                )
            # w2 before the remaining xT blocks: the vW tiles for column
            # block 0 interleave with gT below, so w2 is needed early
            for j in range(NE):
                nc.sync.dma_start(w2_sb[j][:], w2[j * 128 : (j + 1) * 128, :])
            nc.sync.dma_start(ident_sb[:], ident[:, :])
            nc.sync.dma_start(csb_sb[:], csb[:, :])
            for tb in range(1, NB):
                for j in range(NE):
                    nc.sync.dma_start(
                        xT_sb[j][:, tb * TB : (tb + 1) * TB],
                        xT[j * 128 : (j + 1) * 128, tb * TB : (tb + 1) * TB],
                    )
            for j in range(NE):
                nc.sync.dma_start(
                    yT_sb[:, j, :], yT[j * 128 : (j + 1) * 128, :]
                )

            # vW in fp8 k-pair layout for the DoubleRow attn@v matmul:
            # pair tile p holds seq row-tiles (2p, 2p+1) on free dim 0.
            vW_sb = [
                persist.tile([128, 2, E], f8, tag=f"vWs{t}", name=f"vWs{t}")
                for t in range(NT // 2)
            ]

            zbias = persist.tile([128, 1], f32, tag="zbias", name="zbias")
            nc.vector.memset(zbias[:], 0.0)

            # Warm up the PE clock (HAM) during the initial DMA wait: dummy
            # matmuls on a zeroed tile keep TensorE busy so the ~3.4us
            # cold-clock ramp overlaps the input load instead of the first
            # real matmuls.
            if os.environ.get("KERNEL_WARMUP", "1") == "1":
                warm = persist.tile([128, TB], bf16, tag="warm", name="warm")
                nc.vector.memset(warm[:], 0.0)
                for w in range(14):
                    pw = psum_pool.tile(
                        [128, TB], f32, tag="mm", bufs=4, name="pw"
                    )
                    nc.tensor.matmul(
                        pw[:], warm[:, 0:128], warm[:], start=True, stop=True
                    )

            # gT[m][:, tk] = sum_j G[j][:, m-slice].T @ xT[j][:, tk-block]
            # vW[t, :]    = x @ W2   (natural [t, e] layout)
            # Interleaved per xT column block: both only need block tb, so
            # each block gets ~6.6us of compute before the next must arrive —
            # double the DMA slack of a gT-then-vW ordering.
            for tb in range(NB):
                for m in range(NE):
                    ps = psum_pool.tile(
                        [128, TB], f32, tag="mm", bufs=4, name="ps_g"
                    )
                    for j in range(NE):
                        nc.tensor.matmul(
                            ps[:],
                            gw_sb[j][:, m * 128 : (m + 1) * 128],
                            xT_sb[j][:, tb * TB : (tb + 1) * TB],
                            start=(j == 0),
                            stop=(j == NE - 1),
                        )
                    nc.vector.tensor_copy(
                        gT_sb[:, m, tb * TB : (tb + 1) * TB], ps[:]
                    )
                for t in range(4 * tb, 4 * tb + 4):
                    ps = psum_pool.tile([128, E], f32, tag="mm", bufs=4, name="ps_vw")
                    for j in range(NE):
                        nc.tensor.matmul(
                            ps[:],
                            xT_sb[j][:, t * 128 : (t + 1) * 128],
                            w2_sb[j][:],
                            start=(j == 0),
                            stop=(j == NE - 1),
                        )
                    nc.vector.tensor_copy(vW_sb[t // 2][:, t % 2, :], ps[:])

            SCALE = float(E) ** -0.5
            parts = [
                dram.tile([CHUNKS[c], E], f16, tag=f"part{c}", name=f"part{c}")
                for c in range(NCH)
            ]
            rs_outs = [
                dram.tile(
                    [CHUNKS[c] // NCORES, E], f16, tag=f"rso{c}", name=f"rso{c}"
                )
                for c in range(NCH)
            ]
            # (chunk, row-tile within chunk) for each global 128-row tile
            tile2chunk = [
                (c, r) for c, nr in enumerate(CHUNKS) for r in range(nr // 128)
            ]

            for b in range(NB):
                # scoresT[tk, tq-block b] via fp8 DoubleRow (k-pairs on the
                # partition dim; two 256-col halves per PSUM bank), then exp.
                # S = running sum of exp tiles (DVE) for the denominators.
                S = work.tile([128, TB], f32, tag="S", bufs=2, name="S")
                d8_tiles = []
                for tk in range(NT):
                    ps = psum_pool.tile(
                        [128, TB], f32, tag="mm", bufs=4, name="ps_sc"
                    )
                    # h outer: accumulation groups on a shared PSUM bank must
                    # be sequential — an interleaved group's start zeroes the
                    # whole bank on HW, erasing the other group's partials.
                    for h in range(2):
                        for j in range(2):
                            nc.tensor.matmul(
                                ps[:, h * 256 : (h + 1) * 256],
                                gT_sb[:, 2 * j : 2 * j + 2, tk * 128 : (tk + 1) * 128],
                                yT_sb[
                                    :,
                                    2 * j : 2 * j + 2,
                                    b * TB + h * 256 : b * TB + (h + 1) * 256,
                                ],
                                start=(j == 0),
                                stop=(j == 1),
                                perf_mode=mybir.MatmulPerfMode.DoubleRow,
                            )
                    et = expp.tile([128, TB], bf16, tag="expT", bufs=32, name="et")
                    nc.scalar.activation(
                        et[:],
                        ps[:],
                        mybir.ActivationFunctionType.Exp,
                        bias=zbias[:],
                        scale=SCALE,
                    )
                    # delta = exp - M0 in fp8 k-pairs (the out matmul's lhsT);
                    # emitted before the S ops so the out chain unblocks ASAP.
                    if tk % 2 == 0:
                        d8 = expp.tile(
                            [128, 2, TB], f8, tag="d8", bufs=16, name="d8"
                        )
                        d8_tiles.append(d8)
                    nc.vector.tensor_scalar_sub(
                        d8_tiles[tk // 2][:, tk % 2, :], et[:], M0
                    )
                    if tk == 0:
                        nc.vector.tensor_copy(S[:], et[:])
                    else:
                        nc.vector.tensor_add(S[:], S[:], et[:])

                # out rows for this block: accumulate over tk (single
                # 512-wide matmuls), then normalize by 1/denom where
                # denom[tq] comes from transposing S's qi-th 128-col slice.
                for qi in range(TB // 128):
                    pa = psum_pool.tile([128, E], f32, tag="acc", bufs=2, name="pa")
                    # h outer for the same reason: sequential PSUM groups.
                    for h in range(2):
                        for pr in range(NT // 2):
                            nc.tensor.matmul(
                                pa[:, h * 256 : (h + 1) * 256],
                                d8_tiles[pr][:, :, qi * 128 : (qi + 1) * 128],
                                vW_sb[pr][:, :, h * 256 : (h + 1) * 256],
                                start=(pr == 0),
                                stop=(pr == NT // 2 - 1),
                                perf_mode=mybir.MatmulPerfMode.DoubleRow,
                            )
                    pt = psum_pool.tile([128, TB], f32, tag="tr", bufs=2, name="pt")
                    nc.tensor.transpose(
                        pt[:, 0:128], S[:, qi * 128 : (qi + 1) * 128], ident_sb[:]
                    )
                    den = work.tile([128, 1], f32, tag="den", bufs=4, name="den")
                    nc.vector.tensor_reduce(
                        den[:], pt[:, 0:128],
                        mybir.AxisListType.X, mybir.AluOpType.add,
                    )
                    rec = work.tile([128, 1], f32, tag="rec", bufs=4, name="rec")
                    nc.vector.reciprocal(rec[:], den[:])
                    t1 = work.tile([128, E], f32, tag="t1", bufs=4, name="t1")
                    nc.vector.tensor_add(t1[:], pa[:], csb_sb[:])
                    ot = work.tile([128, E], f16, tag="ot", bufs=4, name="ot")
                    nc.vector.tensor_scalar_mul(ot[:], t1[:], rec[:])
                    ch, r = tile2chunk[b * (TB // 128) + qi]
                    nc.sync.dma_start(
                        parts[ch][r * 128 : (r + 1) * 128, :], ot[:]
                    )
                    if r == CHUNKS[ch] // 128 - 1:
                        nc.gpsimd.collective_compute(
                            "ReduceScatter",
                            mybir.AluOpType.add,
                            replica_groups=[list(range(NCORES))],
                            ins=[parts[ch][:]],
                            outs=[rs_outs[ch][:]],
                        )
                        nc.sync.dma_start(out_exts[ch][:], rs_outs[ch][:])

    nc.compile()
    return nc


def kernel(x, y, Wk, Wq, Wv, Wu, bu):
    global last_result
    from concourse.bass_utils import run_bass_kernel_spmd

    if "nc" not in _cache:
        _cache["nc"] = _build_nc()
    nc = _cache["nc"]

    bf = ml_dtypes.bfloat16
    f8 = ml_dtypes.float8_e4m3fn
    x = np.asarray(x, np.float32)
    y = np.asarray(y, np.float32)
    Wk = np.asarray(Wk, np.float32)
    Wq = np.asarray(Wq, np.float32)
    Wv = np.asarray(Wv, np.float32)
    Wu = np.asarray(Wu, np.float32)

    xT = np.ascontiguousarray(x.T).astype(bf)
    yT = np.ascontiguousarray(y.T).astype(f8)
    ident = np.eye(128, dtype=np.float32)

    xsum = x.sum(axis=0)                   # [e] for colsum(vW) = xsum @ W2
    in_maps = []
    for i in range(NCORES):
        sl = slice(i * E, (i + 1) * E)
        G = Wk[:, sl] @ Wq[:, sl].T        # [e, e] fp32 on host
        W2 = Wv[:, sl] @ Wu[sl, :]         # [e, e] fp32 on host
        csb = np.tile((M0 * (xsum @ W2)).astype(np.float32), (128, 1))
        in_maps.append(
            {
                "xT": xT,
                "yT": yT,
                "gw": G.astype(bf),
                "w2": W2.astype(bf),
                "ident": ident,
                "csb": csb,
            }
        )

    trace = os.environ.get("KERNEL_TRACE", "0") == "1"
    res = run_bass_kernel_spmd(
        nc, in_maps, core_ids=list(range(NCORES)), trace=trace
    )
    last_result = res

    out_full = np.empty((T, E), np.float32)
    chunk_r0 = np.cumsum([0] + CHUNKS)[:-1]
    for i in range(NCORES):
        for c in range(NCH):
            nr = CHUNKS[c] // NCORES
            o = np.asarray(res.results[i][f"out{c}"]).astype(np.float32)
            r0 = chunk_r0[c] + i * nr
            out_full[r0 : r0 + nr] = o
    out_full = out_full + np.asarray(bu, np.float32)[None, :]
    return out_full[None]



# revision 7
# speedup vs baseline: 1.1038x; 1.1038x over previous
"""AttentionWide (t=2048, e=512, h=8) on 8 TRN2 NeuronCores.

Tensor-parallel over heads: core i owns head i (columns i*512:(i+1)*512 of
Wk/Wq/Wv, rows i*512:(i+1)*512 of Wu).  Each core computes its head's
attention and the partial unifyheads product; chunked ReduceScatters sum the
partials across cores, each core returning row-shards of the final output.

Weight folding (host-side, exact algebra — the head dim equals emb here so
no information is lost):
    scores = q k^T = (y Wq)(x Wk)^T = y (Wq Wk^T) x^T
      ->  G  = Wk Wq^T   [e, e]   (host)
          gT = G^T xT             (device)
          scoresT[tk, tq] = gT^T yT
    out = attn @ v @ Wu = attn @ (x Wv Wu)
      ->  W2 = Wv Wu     [e, e]   (host)
          vW = x W2               (device)

Device compute in bf16 with fp32 PSUM accumulation; scores and attn@v in
fp8e4 DoubleRow (2x PE).  Differences vs the 173us baseline:
  * softmax denominators on the PE: ones-lhsT DoubleRow matmuls over the
    same d8 (= fp8(exp - M0)) tiles give column sums Sum_tk delta[tk, tq]
    directly; denom = that + 2048*M0.  Removes the per-block DVE running-sum
    chain (16 adds + copy + reduce) that made the attention phase
    Vector-engine-bound (~25us/block DVE vs ~14us/block PE).
  * the M0*colsum(vW) mean-shift correction moved to the host: device
    returns per-core reciprocals (recs), host adds sum_h rec_h (x) csb_h.
  * PE instruction stream software-pipelined: scores(b) interleaved with
    outs(b-1) so the in-order PE queue never stalls on the exp->d8 chain.
  * tiny warm-up ReduceScatter during the input-DMA phase absorbs core
    launch skew + CC-ring cold start before the first real collective.
"""

import os
import numpy as np
import ml_dtypes

T, E, H = 2048, 512, 8
NCORES = 8
TB = 512          # matmul moving-operand block (free dim; one fp32 PSUM bank)
NE = E // 128     # 4  partition tiles of the emb dim
NT = T // 128     # 16 partition tiles of the seq dim
NB = T // TB      # 4  seq blocks
CHUNKS = [768, 768, 512]
NCH = len(CHUNKS)
M0 = 1.02         # mean shift for the fp8 attn@v matmul (exp values ~N(1.02,0.21))

_cache = {}
last_result = None


def _build_nc():
    from concourse import bacc, tile
    from concourse.bass import mybir

    bf16 = mybir.dt.bfloat16
    f16 = mybir.dt.float16
    f32 = mybir.dt.float32
    f8 = mybir.dt.float8e4

    nc = bacc.Bacc(
        "TRN2", target_bir_lowering=False, debug=False, num_devices=NCORES
    )

    xT = nc.dram_tensor("xT", [E, T], bf16, kind="ExternalInput")
    yT = nc.dram_tensor("yT", [E, T], f8, kind="ExternalInput")
    gw = nc.dram_tensor("gw", [E, E], bf16, kind="ExternalInput")   # Wk Wq^T
    w2 = nc.dram_tensor("w2", [E, E], bf16, kind="ExternalInput")   # Wv Wu
    ident = nc.dram_tensor("ident", [128, 128], f32, kind="ExternalInput")
    out_exts = [
        nc.dram_tensor(f"out{c}", [CHUNKS[c] // NCORES, E], f16, kind="ExternalOutput")
        for c in range(NCH)
    ]
    recs_ext = nc.dram_tensor("recs", [128, NT], f32, kind="ExternalOutput")
    warm_ext = nc.dram_tensor("warm", [16, 2], f32, kind="ExternalOutput")

    with tile.TileContext(nc) as tc:
        with (
            tc.tile_pool(name="persist", bufs=1) as persist,
            tc.tile_pool(name="work", bufs=4) as work,
            tc.tile_pool(name="expp", bufs=32) as expp,
            tc.tile_pool(name="psum", bufs=2, space="PSUM") as psum_pool,
            tc.tile_pool(name="dram", bufs=1, space="DRAM") as dram,
        ):
            def alloc_rows(prefix, n):
                return [
                    persist.tile(
                        [128, n], bf16, tag=f"{prefix}{j}", name=f"{prefix}{j}"
                    )
                    for j in range(NE)
                ]

            xT_sb = alloc_rows("xTs", T)
            gw_sb = alloc_rows("gws", E)
            w2_sb = alloc_rows("w2s", E)
            # fp8 wide tiles: free dims = (e-slice, t) so DoubleRow matmuls
            # can take [128, 2, n] k-pair slices.
            yT_sb = persist.tile([128, NE, T], f8, tag="yTs", name="yTs")
            gT_sb = persist.tile([128, NE, T], f8, tag="gTs", name="gTs")
            ident_sb = persist.tile([128, 128], f32, tag="ident", name="ident")

            # DMA order = need order: gw + xT block 0 first (they land on the
            # early-booting first 8 hardware queues), then the rest of xT
            # (gT consumes all of it), yT block 0 (scores b0), w2, yT rest.
            for j in range(NE):
                nc.sync.dma_start(gw_sb[j][:], gw[j * 128 : (j + 1) * 128, :])
                nc.sync.dma_start(
                    xT_sb[j][:, 0:TB], xT[j * 128 : (j + 1) * 128, 0:TB]
                )
            for tb in range(1, NB):
                for j in range(NE):
                    nc.sync.dma_start(
                        xT_sb[j][:, tb * TB : (tb + 1) * TB],
                        xT[j * 128 : (j + 1) * 128, tb * TB : (tb + 1) * TB],
                    )
            for j in range(NE):
                nc.sync.dma_start(
                    yT_sb[:, j, 0:TB], yT[j * 128 : (j + 1) * 128, 0:TB]
                )
            for j in range(NE):
                nc.sync.dma_start(w2_sb[j][:], w2[j * 128 : (j + 1) * 128, :])
            nc.sync.dma_start(ident_sb[:], ident[:, :])
            for tb in range(1, NB):
                for j in range(NE):
                    nc.sync.dma_start(
                        yT_sb[:, j, tb * TB : (tb + 1) * TB],
                        yT[j * 128 : (j + 1) * 128, tb * TB : (tb + 1) * TB],
                    )

            # vW in fp8 k-pair layout for the DoubleRow attn@v matmul:
            # pair tile p holds seq row-tiles (2p, 2p+1) on free dim 0.
            vW_sb = [
                persist.tile([128, 2, E], f8, tag=f"vWs{t}", name=f"vWs{t}")
                for t in range(NT // 2)
            ]

            zbias = persist.tile([128, 1], f32, tag="zbias", name="zbias")
            nc.vector.memset(zbias[:], 0.0)
            # all-ones fp8 k-pair stationary for the denominator matmuls.
            # DoubleRow ldweights needs the k-pair step %16==0 (s3_lw dual-fp8
            # ISA check), so pad the free dim to 16 and slice [:, :, 0:2].
            ones8 = persist.tile([128, 2, 16], f8, tag="ones8", name="ones8")
            nc.vector.memset(ones8[:], 1.0)
            # per-row-tile reciprocals, streamed out at the end for the host
            rec_sb = persist.tile([128, NT], f32, tag="recs", name="recs")

            # Warm-up ReduceScatter: rendezvous the 8 cores and boot the CC
            # rings while the input DMAs land, so the first real RS doesn't
            # absorb launch skew + cold start.
            wtmp = persist.tile([128, 2], f32, tag="wtmp", name="wtmp")
            nc.vector.memset(wtmp[:], 0.0)
            warm_in = dram.tile([128, 2], f32, tag="warm_in", name="warm_in")
            warm_out = dram.tile([16, 2], f32, tag="warm_out", name="warm_out")
            nc.sync.dma_start(warm_in[:], wtmp[:])
            nc.gpsimd.collective_compute(
                "ReduceScatter",
                mybir.AluOpType.add,
                replica_groups=[list(range(NCORES))],
                ins=[warm_in[:]],
                outs=[warm_out[:]],
            )
            nc.sync.dma_start(warm_ext[:], warm_out[:])

            # Warm up the PE clock (HAM) during the initial DMA wait: dummy
            # matmuls on a zeroed tile keep TensorE busy so the ~3.4us
            # cold-clock ramp overlaps the input load instead of the first
            # real matmuls.
            warm = persist.tile([128, TB], bf16, tag="warm", name="warm")
            nc.vector.memset(warm[:], 0.0)
            for w in range(14):
                pw = psum_pool.tile(
                    [128, TB], f32, tag="mm", bufs=4, name="pw"
                )
                nc.tensor.matmul(
                    pw[:], warm[:, 0:128], warm[:], start=True, stop=True
                )

            # ---- projections ----
            # gT[m][:, tk] = sum_j G[j][:, m-slice].T @ xT[j][:, tk-block]
            # All of gT first (scores for block 0 need every tk column);
            # PSUM->fp8 copies on the otherwise-idle Scalar engine.
            for tb in range(NB):
                for m in range(NE):
                    ps = psum_pool.tile(
                        [128, TB], f32, tag="mm", bufs=4, name="ps_g"
                    )
                    for j in range(NE):
                        nc.tensor.matmul(
                            ps[:],
                            gw_sb[j][:, m * 128 : (m + 1) * 128],
                            xT_sb[j][:, tb * TB : (tb + 1) * TB],
                            start=(j == 0),
                            stop=(j == NE - 1),
                        )
                    nc.scalar.copy(gT_sb[:, m, tb * TB : (tb + 1) * TB], ps[:])

            SCALE = float(E) ** -0.5
            parts = [
                dram.tile([CHUNKS[c], E], f16, tag=f"part{c}", name=f"part{c}")
                for c in range(NCH)
            ]
            rs_outs = [
                dram.tile(
                    [CHUNKS[c] // NCORES, E], f16, tag=f"rso{c}", name=f"rso{c}"
                )
                for c in range(NCH)
            ]
            # (chunk, row-tile within chunk) for each global 128-row tile
            tile2chunk = [
                (c, r) for c, nr in enumerate(CHUNKS) for r in range(nr // 128)
            ]

            d8_blocks = [None] * NB

            def emit_scores_tile(b, tk):
                """scoresT[tk-tile, tq-block b] -> exp -> d8 fp8 k-pairs."""
                ps = psum_pool.tile(
                    [128, TB], f32, tag="mm", bufs=4, name="ps_sc"
                )
                # h outer: accumulation groups on a shared PSUM bank must
                # be sequential — an interleaved group's start zeroes the
                # whole bank on HW, erasing the other group's partials.
                for h in range(2):
                    for j in range(2):
                        nc.tensor.matmul(
                            ps[:, h * 256 : (h + 1) * 256],
                            gT_sb[:, 2 * j : 2 * j + 2, tk * 128 : (tk + 1) * 128],
                            yT_sb[
                                :,
                                2 * j : 2 * j + 2,
                                b * TB + h * 256 : b * TB + (h + 1) * 256,
                            ],
                            start=(j == 0),
                            stop=(j == 1),
                            perf_mode=mybir.MatmulPerfMode.DoubleRow,
                        )
                et = expp.tile([128, TB], bf16, tag="expT", bufs=32, name="et")
                nc.scalar.activation(
                    et[:],
                    ps[:],
                    mybir.ActivationFunctionType.Exp,
                    bias=zbias[:],
                    scale=SCALE,
                )
                if tk % 2 == 0:
                    d8 = expp.tile(
                        [128, 2, TB], f8, tag="d8", bufs=16, name="d8"
                    )
                    d8_blocks[b].append(d8)
                nc.vector.tensor_scalar_sub(
                    d8_blocks[b][tk // 2][:, tk % 2, :], et[:], M0
                )

            def emit_vw_tile(t):
                """vW[t-tile, :] = x @ W2 (natural [t, e] layout), fp8."""
                ps = psum_pool.tile([128, E], f32, tag="mm", bufs=4, name="ps_vw")
                for j in range(NE):
                    nc.tensor.matmul(
                        ps[:],
                        xT_sb[j][:, t * 128 : (t + 1) * 128],
                        w2_sb[j][:],
                        start=(j == 0),
                        stop=(j == NE - 1),
                    )
                nc.vector.tensor_copy(vW_sb[t // 2][:, t % 2, :], ps[:])

            def emit_denoms(b):
                """denom column sums on the PE: Sum_tk d8[tk, tq] for block b
                via ones-lhsT DoubleRow matmuls, then +2048*M0 into SBUF."""
                d8s = d8_blocks[b]
                pd = psum_pool.tile([2, TB], f32, tag="den", bufs=1, name="pd")
                for c in range(2):
                    for pr in range(NT // 2):
                        nc.tensor.matmul(
                            pd[0:2, c * 256 : (c + 1) * 256],
                            ones8[:, :, 0:2],
                            d8s[pr][:, :, c * 256 : (c + 1) * 256],
                            start=(pr == 0),
                            stop=(pr == NT // 2 - 1),
                            perf_mode=mybir.MatmulPerfMode.DoubleRow,
                        )
                den = work.tile([1, TB], f32, tag="den_sb", bufs=2, name="den")
                nc.vector.tensor_scalar_add(den[0:1, :], pd[0:1, :], float(T) * M0)
                return den

            def emit_out_qi(b, qi, den):
                """out rows [128] for (block b, qi): attn@vW + normalize."""
                d8s = d8_blocks[b]
                g = b * (TB // 128) + qi
                pa = psum_pool.tile([128, E], f32, tag="acc", bufs=2, name="pa")
                for h in range(2):
                    for pr in range(NT // 2):
                        nc.tensor.matmul(
                            pa[:, h * 256 : (h + 1) * 256],
                            d8s[pr][:, :, qi * 128 : (qi + 1) * 128],
                            vW_sb[pr][:, :, h * 256 : (h + 1) * 256],
                            start=(pr == 0),
                            stop=(pr == NT // 2 - 1),
                            perf_mode=mybir.MatmulPerfMode.DoubleRow,
                        )
                # transpose den [1,128] -> [128,1]; rhs is a 1x1 identity
                pt = psum_pool.tile([128, 1], f32, tag="tr", bufs=1, name="pt")
                nc.tensor.transpose(
                    pt[:, 0:1],
                    den[0:1, qi * 128 : (qi + 1) * 128],
                    ident_sb[0:1, 0:1],
                )
                nc.vector.reciprocal(rec_sb[:, g : g + 1], pt[:, 0:1])
                ot = work.tile([128, E], f16, tag="ot", bufs=4, name="ot")
                nc.vector.tensor_scalar_mul(ot[:], pa[:], rec_sb[:, g : g + 1])
                ch, r = tile2chunk[g]
                nc.sync.dma_start(parts[ch][r * 128 : (r + 1) * 128, :], ot[:])
                if r == CHUNKS[ch] // 128 - 1:
                    nc.gpsimd.collective_compute(
                        "ReduceScatter",
                        mybir.AluOpType.add,
                        replica_groups=[list(range(NCORES))],
                        ins=[parts[ch][:]],
                        outs=[rs_outs[ch][:]],
                    )
                    nc.sync.dma_start(out_exts[ch][:], rs_outs[ch][:])

            # ---- attention, software-pipelined over blocks ----
            # b0: scores interleaved with the vW projection tiles.
            d8_blocks[0] = []
            for s in range(4):
                for tk in range(4 * s, 4 * s + 4):
                    emit_scores_tile(0, tk)
                for t in range(4 * s, 4 * s + 4):
                    emit_vw_tile(t)
            # steady state: scores(b) interleaved with denoms+outs(b-1).
            for b in range(1, NB):
                d8_blocks[b] = []
                den = emit_denoms(b - 1)
                for s in range(4):
                    for tk in range(4 * s, 4 * s + 4):
                        emit_scores_tile(b, tk)
                    emit_out_qi(b - 1, s, den)
                d8_blocks[b - 1] = None
            # tail: outs for the last block.
            den = emit_denoms(NB - 1)
            for qi in range(4):
                emit_out_qi(NB - 1, qi, den)
            nc.sync.dma_start(recs_ext[:], rec_sb[:])

    nc.compile()
    return nc


def kernel(x, y, Wk, Wq, Wv, Wu, bu):
    global last_result
    from concourse.bass_utils import run_bass_kernel_spmd

    if "nc" not in _cache:
        _cache["nc"] = _build_nc()
    nc = _cache["nc"]

    bf = ml_dtypes.bfloat16
    f8 = ml_dtypes.float8_e4m3fn
    x = np.asarray(x, np.float32)
    y = np.asarray(y, np.float32)
    Wk = np.asarray(Wk, np.float32)
    Wq = np.asarray(Wq, np.float32)
    Wv = np.asarray(Wv, np.float32)
    Wu = np.asarray(Wu, np.float32)

    xT = np.ascontiguousarray(x.T).astype(bf)
    yT = np.ascontiguousarray(y.T).astype(f8)
    ident = np.eye(128, dtype=np.float32)

    xsum = x.sum(axis=0)                   # [e] for colsum(vW) = xsum @ W2
    in_maps = []
    csb_rows = []
    for i in range(NCORES):
        sl = slice(i * E, (i + 1) * E)
        G = Wk[:, sl] @ Wq[:, sl].T        # [e, e] fp32 on host
        W2 = Wv[:, sl] @ Wu[sl, :]         # [e, e] fp32 on host
        csb_rows.append((M0 * (xsum @ W2)).astype(np.float32))
        in_maps.append(
            {
                "xT": xT,
                "yT": yT,
                "gw": G.astype(bf),
                "w2": W2.astype(bf),
                "ident": ident,
            }
        )

    trace = os.environ.get("KERNEL_TRACE", "0") == "1"
    res = run_bass_kernel_spmd(
        nc, in_maps, core_ids=list(range(NCORES)), trace=trace
    )
    last_result = res

    out_full = np.empty((T, E), np.float32)
    chunk_r0 = np.cumsum([0] + CHUNKS)[:-1]
    for i in range(NCORES):
        for c in range(NCH):
            nr = CHUNKS[c] // NCORES
            o = np.asarray(res.results[i][f"out{c}"]).astype(np.float32)
            r0 = chunk_r0[c] + i * nr
            out_full[r0 : r0 + nr] = o
    # host-side mean-shift correction: sum_h rec_h (outer) csb_h, + bias
    R = np.stack(
        [
            np.asarray(res.results[i]["recs"], np.float32).T.reshape(T)
            for i in range(NCORES)
        ],
        axis=1,
    )                                       # [T, NCORES]
    C = np.stack(csb_rows, axis=0)          # [NCORES, e]
    out_full = out_full + R @ C + np.asarray(bu, np.float32)[None, :]
    return out_full[None]
